# revision 1
# baseline (speedup 1.0000x reference)
"""Converse2D-Up (FFT deconvolution upsampler) as a Bass/Tile kernel for TRN2.

Math (validated against the jax reference to rel-l2 ~1.1e-4 == the
reference's own fp32 noise floor):

The whole pipeline before the final gelu is linear in x and channel-wise.
With xp = wrap-pad(x) (132x132), Y = FFT132(xp) = G @ x @ G^T where
G = F132 @ P (132x128, P = periodic pad selection).  The reference's
264-point spectral transfer function H (built from weight/bias only) is
Hermitian, so out = crop(real(IFFT264(H . tile(Y)))) decomposes into 4
polyphase outputs out_dd = real(IFFT132(Kdd_hat . Y)) with per-channel
precomputed spectra Kdd_hat; the crop leaves exactly 128 rows/cols per
phase.  Hermitian symmetry further means only columns v=0..66 of
Kdd_hat.Y are needed:
    T1[x,v] = sum_u Ai[x,u] (Kdd_hat.Y)[u,v]      (Ai = cropped iF132 rows)
    out[x,y] = sum_{v=0..66} w_v Re(T1[x,v] Ai[y,v]),  w = [1,2,...,2,1]
Everything maps onto fp32 PE matmuls with K<=132 contractions (split
128+4), a small pointwise complex multiply (DVE for the 128-row chunk,
GPSIMD for the 4-row chunk), and gelu+phase-interleave fused into the
ScalarE PSUM->SBUF eviction.

Sharding: 8 channels per core x 4 batch images (all per-(B,C)
independent); weight/bias-derived spectra are host-precomputed constants.
"""

import os

import numpy as np

import concourse.bass as bass
import concourse.mybir as mybir
import concourse.tile as tile
from concourse import bacc
from concourse.bass import ts
from concourse.bass_utils import run_bass_kernel_spmd

F32 = mybir.dt.float32
AF = mybir.ActivationFunctionType

SCALE = 2
PAD = 2
EPS = 1e-5
N0 = 128           # input spatial size
NP = N0 + 2 * PAD  # 132 padded
NU = NP * SCALE    # 264 upsampled
NV = NP // 2 + 1   # 67 unique spectral columns
B = 4
C = 64
NCORES = 8
CPC = C // NCORES  # 8 channels per core
NIMG = B * CPC     # 32 images per core

LAST_EXEC_NS = None  # set by kernel() when tracing is enabled


# --------------------------------------------------------------------------
# host-side constant precompute (weight/bias -> per-channel spectra)
# --------------------------------------------------------------------------

def _host_constants(weight, bias):
    w64 = np.asarray(weight, dtype=np.float64)
    b64 = np.asarray(bias, dtype=np.float64)

    # FB = p2o(weight): 264-point OTF of the rolled 3x3 PSF, per channel
    k_h, k_w = w64.shape[-2:]
    otf = np.zeros((C, NU, NU), dtype=np.complex128)
    otf[:, :k_h, :k_w] = w64[0]
    otf = np.roll(otf, (-(k_h // 2), -(k_w // 2)), axis=(-2, -1))
    FB = np.fft.fftn(otf, axes=(-2, -1))                      # (C,264,264)

    biaseps = 1.0 / (1.0 + np.exp(-(b64.reshape(C) - 9.0))) + EPS  # (C,)
    be = biaseps[:, None, None]

    u = np.arange(NU)
    Dr = 1 + np.exp(-2j * np.pi * u / NU)
    D = Dr[:, None] * Dr[None, :]                             # (264,264)

    Gh = np.conj(FB) + be * D[None]
    FBG = FB * Gh

    def quadmean(A):
        return 0.25 * (A[:, :NP, :NP] + A[:, NP:, :NP]
                       + A[:, :NP, NP:] + A[:, NP:, NP:])

    M1 = quadmean(FBG)
    invW = quadmean(np.abs(FB) ** 2)
    M2 = M1 / (invW + be)
    H = (Gh - np.conj(FB) * np.tile(M2, (1, SCALE, SCALE))) / be   # (C,264,264)

    hr = np.fft.ifft2(H, axes=(-2, -1)).real                  # H Hermitian
    # polyphase spectra: Kdd_hat[c,dx,dy] = FFT132(hr[c, dx::2, dy::2])
    kdd = np.empty((C, 2, 2, NP, NV), dtype=np.complex128)
    for dx in range(2):
        for dy in range(2):
            kh = np.fft.fft2(hr[:, dx::2, dy::2], axes=(-2, -1))
            kdd[:, dx, dy] = kh[:, :, :NV]

    # pack per channel: [u, plane(3), phase(4), v] planes = [Kr, Ki, Kr]
    kr = np.empty((C, NP, 4, NV), dtype=np.float32)
    ki = np.empty((C, NP, 4, NV), dtype=np.float32)
    for dx in range(2):
        for dy in range(2):
            p = dx * 2 + dy
            kr[:, :, p, :] = kdd[:, dx, dy].real.astype(np.float32)
            ki[:, :, p, :] = kdd[:, dx, dy].imag.astype(np.float32)
    kdd_packed = np.concatenate(
        [kr.reshape(C, NP, 4 * NV), ki.reshape(C, NP, 4 * NV),
         kr.reshape(C, NP, 4 * NV)], axis=2,
    )                                                          # (C,132,804)

    # forward matrix G = F132 @ P  (132x128 complex)
    P = np.zeros((NP, N0))
    for m in range(NP):
        P[m, (m - PAD) % N0] = 1.0
    F132 = np.exp(-2j * np.pi * np.outer(np.arange(NP), np.arange(NP)) / NP)
    G = F132 @ P

    gt = np.concatenate([G.real.T, G.imag.T], axis=1).astype(np.float32)   # (128,264)
    neg67 = (-G.imag.T[:, :NV]).astype(np.float32)                          # (128,67)

    # inverse matrix, rows i in [2,130) of iF132/132
    Ai = np.exp(2j * np.pi * np.outer(np.arange(2, 130), np.arange(NP)) / NP) / NP
    Cm, Sm = Ai.real, Ai.imag                                  # (128,132)
    CT, ST = Cm.T, Sm.T                                        # (132,128)
    cst = np.concatenate([CT, ST, -ST], axis=1).astype(np.float32)  # (132,384)

    w_v = np.ones(NV)
    w_v[1:NV - 1] = 2.0
    RC = (Cm[:, :NV] * w_v[None, :]).T.astype(np.float32)      # (67,128)
    RS = (-Sm[:, :NV] * w_v[None, :]).T.astype(np.float32)
    rcs = np.concatenate([RC, RS], axis=1).astype(np.float32)  # (67,256)

    return {
        "kdd_packed": kdd_packed.astype(np.float32),
        "gt": gt,
        "neg67": neg67,
        "cst_hi": np.ascontiguousarray(cst[:128]),
        "cst_lo": np.ascontiguousarray(cst[128:]),
        "rcs": rcs,
    }


# --------------------------------------------------------------------------
# device kernel
# --------------------------------------------------------------------------

def build_nc(n_chan=CPC, n_batch=B, gelu=True):
    act_fn = AF.Gelu if gelu else AF.Copy
    n_img = n_chan * n_batch
    nc = bacc.Bacc("TRN2", target_bir_lowering=False, debug=False,
                   enable_asserts=False)

    x_t = nc.dram_tensor("x", [n_img, N0, N0], F32, kind="ExternalInput")
    kdd_t = nc.dram_tensor("kdd", [n_chan, NP, 3 * 4 * NV], F32,
                           kind="ExternalInput")
    gt_t = nc.dram_tensor("gt", [128, 2 * NP], F32, kind="ExternalInput")
    neg67_t = nc.dram_tensor("neg67", [128, NV], F32, kind="ExternalInput")
    csth_t = nc.dram_tensor("cst_hi", [128, 384], F32, kind="ExternalInput")
    cstl_t = nc.dram_tensor("cst_lo", [4, 384], F32, kind="ExternalInput")
    rcs_t = nc.dram_tensor("rcs", [NV, 256], F32, kind="ExternalInput")
    out_t = nc.dram_tensor("out", [n_img, 2 * N0, 2 * N0], F32,
                           kind="ExternalOutput")

    PH4 = 4 * NV          # 268
    with tile.TileContext(nc) as tc:
        with (
            tc.tile_pool(name="consts", bufs=1) as cpool,
            tc.tile_pool(name="kdd", bufs=2) as kpool,
            tc.tile_pool(name="xin", bufs=3) as xpool,
            tc.tile_pool(name="r1", bufs=2) as r1pool,
            tc.tile_pool(name="ylo", bufs=2) as ylopool,
            tc.tile_pool(name="prod", bufs=2) as prodpool,
            tc.tile_pool(name="fx", bufs=2) as fxpool,
            tc.tile_pool(name="t1", bufs=2) as t1pool,
            tc.tile_pool(name="osb", bufs=2) as opool,
            tc.tile_pool(name="ppa", bufs=2, space="PSUM") as ppa_pool,
            tc.tile_pool(name="ppy", bufs=2, space="PSUM") as ppy_pool,
            tc.tile_pool(name="pt1", bufs=1, space="PSUM") as pt1_pool,
            tc.tile_pool(name="ppd", bufs=2, space="PSUM") as ppd_pool,
        ):
            gt = cpool.tile([128, 2 * NP], F32)
            nc.sync.dma_start(gt[:], gt_t[:])
            neg67 = cpool.tile([128, NV], F32)
            nc.sync.dma_start(neg67[:], neg67_t[:])
            cst_hi = cpool.tile([128, 384], F32)
            nc.sync.dma_start(cst_hi[:], csth_t[:])
            cst_lo = cpool.tile([4, 384], F32)
            nc.sync.dma_start(cst_lo[:], cstl_t[:])
            rcs = cpool.tile([NV, 256], F32)
            nc.sync.dma_start(rcs[:], rcs_t[:])

            for ci in range(n_chan):
                k_hi = kpool.tile([128, 3 * PH4], F32, tag="k_hi")
                nc.sync.dma_start(k_hi[:], kdd_t[ci, 0:128])
                k_lo = kpool.tile([4, 3 * PH4], F32, tag="k_lo")
                nc.sync.dma_start(k_lo[:], kdd_t[ci, 128:NP])

                for bi in range(n_batch):
                    img = ci * n_batch + bi

                    x_tile = xpool.tile([N0, N0], F32, tag="x")
                    nc.sync.dma_start(x_tile[:], x_t[img])

                    # ---- stage A: R1^T = x^T @ [Gr^T | Gi^T]  (PSUM) ----
                    pA = ppa_pool.tile([128, 2 * NP], F32, tag="pA")
                    nc.tensor.matmul(pA[:], x_tile[:], gt[:],
                                     start=True, stop=True)
                    r1 = r1pool.tile([128, 2 * NP], F32, tag="r1")
                    nc.scalar.activation(r1[:], pA[:], AF.Copy)

                    # ---- stage B: Y = R1 @ G^T, cols 0..66 ----
                    # pY layout: [:,0:67]=Yr_hi  [:,67:134]=Yi_hi
                    #            [0:4,134:201]=Yr_lo  [0:4,201:268]=Yi_lo
                    pY = ppy_pool.tile([128, PH4], F32, tag="pY")
                    nc.tensor.matmul(pY[:, 0:NV], r1[:, 0:128],
                                     gt[:, 0:NV], start=True, stop=False)
                    nc.tensor.matmul(pY[:, 0:NV], r1[:, NP:NP + 128],
                                     neg67[:], start=False, stop=True)
                    nc.tensor.matmul(pY[:, NV:2 * NV], r1[:, 0:128],
                                     gt[:, NP:NP + NV], start=True, stop=False)
                    nc.tensor.matmul(pY[:, NV:2 * NV], r1[:, NP:NP + 128],
                                     gt[:, 0:NV], start=False, stop=True)
                    nc.tensor.matmul(pY[0:4, 2 * NV:3 * NV], r1[:, 128:NP],
                                     gt[:, 0:NV], start=True, stop=False)
                    nc.tensor.matmul(pY[0:4, 2 * NV:3 * NV], r1[:, NP + 128:2 * NP],
                                     neg67[:], start=False, stop=True)
                    nc.tensor.matmul(pY[0:4, 3 * NV:4 * NV], r1[:, 128:NP],
                                     gt[:, NP:NP + NV], start=True, stop=False)
                    nc.tensor.matmul(pY[0:4, 3 * NV:4 * NV], r1[:, NP + 128:2 * NP],
                                     gt[:, 0:NV], start=False, stop=True)

                    # Y lo rows to SBUF for gpsimd (gpsimd cannot read PSUM)
                    ylo = ylopool.tile([4, 2 * NV], F32, tag="ylo")
                    nc.scalar.activation(ylo[:], pY[0:4, 2 * NV:4 * NV], AF.Copy)

                    # ---- FX = Kdd_hat * Y, per phase (pointwise cmul) ----
                    # hi rows on DVE, reading Y straight from PSUM
                    y_hi_b = (pY[:, 0:2 * NV]
                              .rearrange("p (a v) -> p a v", a=2)
                              [:, :, None, :]
                              .broadcast_to([128, 2, 4, NV]))
                    pa_hi = prodpool.tile([128, 2 * PH4], F32, tag="pa_hi")
                    nc.vector.tensor_mul(
                        pa_hi[:].rearrange("p (a f v) -> p a f v", a=2, f=4),
                        k_hi[:, 0:2 * PH4].rearrange("p (a f v) -> p a f v",
                                                     a=2, f=4),
                        y_hi_b)
                    pb_hi = prodpool.tile([128, 2 * PH4], F32, tag="pb_hi")
                    nc.vector.tensor_mul(
                        pb_hi[:].rearrange("p (a f v) -> p a f v", a=2, f=4),
                        k_hi[:, PH4:3 * PH4].rearrange("p (a f v) -> p a f v",
                                                       a=2, f=4),
                        y_hi_b)
                    fxr_hi = fxpool.tile([128, PH4], F32, tag="fxr_hi")
                    nc.vector.tensor_sub(fxr_hi[:], pa_hi[:, 0:PH4],
                                         pa_hi[:, PH4:2 * PH4])
                    fxi_hi = fxpool.tile([128, PH4], F32, tag="fxi_hi")
                    nc.vector.tensor_add(fxi_hi[:], pb_hi[:, 0:PH4],
                                         pb_hi[:, PH4:2 * PH4])

                    # lo rows (u=128..131) on GPSIMD
                    y_lo_b = (ylo[:]
                              .rearrange("p (a v) -> p a v", a=2)
                              [:, :, None, :]
                              .broadcast_to([4, 2, 4, NV]))
                    pa_lo = prodpool.tile([4, 2 * PH4], F32, tag="pa_lo")
                    nc.gpsimd.tensor_mul(
                        pa_lo[:].rearrange("p (a f v) -> p a f v", a=2, f=4),
                        k_lo[:, 0:2 * PH4].rearrange("p (a f v) -> p a f v",
                                                     a=2, f=4),
                        y_lo_b)
                    pb_lo = prodpool.tile([4, 2 * PH4], F32, tag="pb_lo")
                    nc.gpsimd.tensor_mul(
                        pb_lo[:].rearrange("p (a f v) -> p a f v", a=2, f=4),
                        k_lo[:, PH4:3 * PH4].rearrange("p (a f v) -> p a f v",
                                                       a=2, f=4),
                        y_lo_b)
                    fxr_lo = fxpool.tile([4, PH4], F32, tag="fxr_lo")
                    nc.gpsimd.tensor_sub(fxr_lo[:], pa_lo[:, 0:PH4],
                                         pa_lo[:, PH4:2 * PH4])
                    fxi_lo = fxpool.tile([4, PH4], F32, tag="fxi_lo")
                    nc.gpsimd.tensor_add(fxi_lo[:], pb_lo[:, 0:PH4],
                                         pb_lo[:, PH4:2 * PH4])

                    # ---- stage C': T1^T[v,x] per phase (PSUM [67,512]) ----
                    pT1r = pt1_pool.tile([NV, 512], F32, tag="pT1r")
                    pT1i = pt1_pool.tile([NV, 512], F32, tag="pT1i")
                    for p in range(4):
                        o = pT1r[:, ts(p, 128)]
                        nc.tensor.matmul(o, fxr_hi[:, ts(p, NV)],
                                         cst_hi[:, 0:128], start=True, stop=False)
                        nc.tensor.matmul(o, fxi_hi[:, ts(p, NV)],
                                         cst_hi[:, 256:384], start=False, stop=False)
                        nc.tensor.matmul(o, fxr_lo[:, ts(p, NV)],
                                         cst_lo[:, 0:128], start=False, stop=False)
                        nc.tensor.matmul(o, fxi_lo[:, ts(p, NV)],
                                         cst_lo[:, 256:384], start=False, stop=True)
                        o = pT1i[:, ts(p, 128)]
                        nc.tensor.matmul(o, fxi_hi[:, ts(p, NV)],
                                         cst_hi[:, 0:128], start=True, stop=False)
                        nc.tensor.matmul(o, fxr_hi[:, ts(p, NV)],
                                         cst_hi[:, 128:256], start=False, stop=False)
                        nc.tensor.matmul(o, fxi_lo[:, ts(p, NV)],
                                         cst_lo[:, 0:128], start=False, stop=False)
                        nc.tensor.matmul(o, fxr_lo[:, ts(p, NV)],
                                         cst_lo[:, 128:256], start=False, stop=True)

                    t1sb = t1pool.tile([NV, 1024], F32, tag="t1sb")
                    nc.scalar.activation(t1sb[:, 0:512], pT1r[:], AF.Copy)
                    nc.scalar.activation(t1sb[:, 512:1024], pT1i[:], AF.Copy)

                    # ---- stage D: out_p = T1r@RC + T1i@RS  (PSUM [128,512]) ----
                    pD = ppd_pool.tile([128, 512], F32, tag="pD")
                    for p in range(4):
                        o = pD[:, ts(p, 128)]
                        nc.tensor.matmul(o, t1sb[:, ts(p, 128)],
                                         rcs[:, 0:128], start=True, stop=False)
                        nc.tensor.matmul(o, t1sb[:, 512 + p * 128:512 + (p + 1) * 128],
                                         rcs[:, 128:256], start=False, stop=True)

                    # ---- gelu + phase interleave + store ----
                    oute = opool.tile([128, 256], F32, tag="oute")
                    outo = opool.tile([128, 256], F32, tag="outo")
                    nc.scalar.activation(
                        oute[:].rearrange("p (v d) -> p d v", d=2),
                        pD[:, 0:256].rearrange("p (d v) -> p d v", d=2),
                        act_fn)
                    nc.scalar.activation(
                        outo[:].rearrange("p (v d) -> p d v", d=2),
                        pD[:, 256:512].rearrange("p (d v) -> p d v", d=2),
                        act_fn)
                    orows = out_t[img].rearrange("(x d) y -> d x y", d=2)
                    nc.sync.dma_start(orows[0], oute[:])
                    nc.sync.dma_start(orows[1], outo[:])

    nc.compile()
    return nc


# --------------------------------------------------------------------------
# public entry point: full inputs in, full output out
# --------------------------------------------------------------------------

def kernel(x, weight, bias):
    global LAST_EXEC_NS
    x = np.ascontiguousarray(np.asarray(x, dtype=np.float32))
    consts = _host_constants(weight, bias)

    nc = build_nc()

    in_maps = []
    for core in range(NCORES):
        c0 = core * CPC
        xs = np.ascontiguousarray(
            x[:, c0:c0 + CPC].transpose(1, 0, 2, 3)).reshape(NIMG, N0, N0)
        in_maps.append({
            "x": xs,
            "kdd": np.ascontiguousarray(consts["kdd_packed"][c0:c0 + CPC]),
            "gt": consts["gt"],
            "neg67": consts["neg67"],
            "cst_hi": consts["cst_hi"],
            "cst_lo": consts["cst_lo"],
            "rcs": consts["rcs"],
        })

    trace = os.environ.get("KERNEL_TRACE", "0") == "1"
    tmpdir = os.environ.get("KERNEL_TMPDIR") or None
    res = run_bass_kernel_spmd(nc, in_maps, list(range(NCORES)), trace=trace,
                               tmpdir=tmpdir)
    LAST_EXEC_NS = res.exec_time_ns

    out = np.empty((B, C, 2 * N0, 2 * N0), dtype=np.float32)
    for core in range(NCORES):
        c0 = core * CPC
        o = res.results[core]["out"].reshape(CPC, B, 2 * N0, 2 * N0)
        out[:, c0:c0 + CPC] = o.transpose(1, 0, 2, 3)
    return out



# revision 5
# speedup vs baseline: 2.5395x; 2.5395x over previous
"""Converse2D-Up (FFT deconvolution upsampler) as a Bass/Tile kernel for TRN2.

Restructured for f32r matmuls (FP22 multiply, 1 cyc/row at N>=256 vs
fp32's 4 cyc/row + 2 HW passes).  Dataflow per image (validated in numpy
against the jax reference to rel-l2 ~1.1e-4):

  A: r1[y,u_ri] = x^T @ [Gr^T|Gi^T]           (stat=x, N=264)
  B: Y^T[v,u] per img-pair, const-stationary   (stat=G-cols, N=264)
  T: 4 PE transposes Y^T -> Y[u,v] hi/lo       (fp32, small)
  FX: complex mult K (.) Y on DVE              ([u,(p,v)] layout)
  C: T1^T[v,(x|x)] = FX-stat @ [C|S] consts    (N=256, lo rows merged
     into one K=8 matmul via [[FXr_lo];[FXi_lo]] stacking)
  D: out_p[x,(y|junk)] = T1-stat @ [RC|RS]     (N=256)
  gelu + phase interleave on ScalarE eviction, as before.

Sharding: 8 channels per core x 4 batch images; weight/bias spectra are
host-precomputed constants.
"""

import os

import numpy as np

import concourse.bass as bass
import concourse.mybir as mybir
import concourse.tile as tile
from concourse import bacc
from concourse.bass_utils import run_bass_kernel_spmd

F32 = mybir.dt.float32
F32R = mybir.dt.float32r
AF = mybir.ActivationFunctionType

SCALE = 2
PAD = 2
EPS = 1e-5
N0 = 128           # input spatial size
NP = N0 + 2 * PAD  # 132 padded
NU = NP * SCALE    # 264 upsampled
NV = NP // 2 + 1   # 67 unique spectral columns
B = 4
C = 64
NCORES = 8
CPC = C // NCORES  # 8 channels per core
NIMG = B * CPC     # 32 images per core
PH4 = 4 * NV       # 268

LAST_EXEC_NS = None  # set by kernel() when tracing is enabled


# --------------------------------------------------------------------------
# host-side constant precompute (weight/bias -> per-channel spectra)
# --------------------------------------------------------------------------

def _host_constants(weight, bias):
    w64 = np.asarray(weight, dtype=np.float64)
    b64 = np.asarray(bias, dtype=np.float64)

    k_h, k_w = w64.shape[-2:]
    otf = np.zeros((C, NU, NU), dtype=np.complex128)
    otf[:, :k_h, :k_w] = w64[0]
    otf = np.roll(otf, (-(k_h // 2), -(k_w // 2)), axis=(-2, -1))
    FB = np.fft.fftn(otf, axes=(-2, -1))                      # (C,264,264)

    biaseps = 1.0 / (1.0 + np.exp(-(b64.reshape(C) - 9.0))) + EPS
    be = biaseps[:, None, None]

    u = np.arange(NU)
    Dr = 1 + np.exp(-2j * np.pi * u / NU)
    D = Dr[:, None] * Dr[None, :]

    Gh = np.conj(FB) + be * D[None]
    FBG = FB * Gh

    def quadmean(A):
        return 0.25 * (A[:, :NP, :NP] + A[:, NP:, :NP]
                       + A[:, :NP, NP:] + A[:, NP:, NP:])

    M1 = quadmean(FBG)
    invW = quadmean(np.abs(FB) ** 2)
    M2 = M1 / (invW + be)
    H = (Gh - np.conj(FB) * np.tile(M2, (1, SCALE, SCALE))) / be

    hr = np.fft.ifft2(H, axes=(-2, -1)).real
    kdd = np.empty((C, 2, 2, NP, NV), dtype=np.complex128)
    for dx in range(2):
        for dy in range(2):
            kh = np.fft.fft2(hr[:, dx::2, dy::2], axes=(-2, -1))
            kdd[:, dx, dy] = kh[:, :, :NV]

    # K tiles in [u, (ri, p, v)] layout
    khi = np.empty((C, 128, 2 * PH4), dtype=np.float32)
    klo = np.empty((C, 4, 2 * PH4), dtype=np.float32)
    for dx in range(2):
        for dy in range(2):
            p = dx * 2 + dy
            kr = kdd[:, dx, dy].real
            ki = kdd[:, dx, dy].imag
            khi[:, :, p * NV:(p + 1) * NV] = kr[:, :128]
            khi[:, :, PH4 + p * NV:PH4 + (p + 1) * NV] = ki[:, :128]
            klo[:, :, p * NV:(p + 1) * NV] = kr[:, 128:]
            klo[:, :, PH4 + p * NV:PH4 + (p + 1) * NV] = ki[:, 128:]

    # forward matrix G = F132 @ P  (132x128 complex)
    P = np.zeros((NP, N0))
    for m in range(NP):
        P[m, (m - PAD) % N0] = 1.0
    F132 = np.exp(-2j * np.pi * np.outer(np.arange(NP), np.arange(NP)) / NP)
    G = F132 @ P
    gt = np.concatenate([G.real.T, G.imag.T], axis=1).astype(np.float32)

    # stage-B stationaries: [Gr_v | -Gi_v | Gi_v]  (128, 201)
    gtv = np.concatenate([gt[:, 0:NV], -gt[:, NP:NP + NV],
                          gt[:, NP:NP + NV]], axis=1).astype(np.float32)

    # inverse rows i in [2,130) of iF132/132
    Ai = np.exp(2j * np.pi * np.outer(np.arange(2, 130), np.arange(NP)) / NP) / NP
    Cm, Sm = Ai.real, Ai.imag
    CT = Cm.T.astype(np.float32)                               # (132,128)
    ST = Sm.T.astype(np.float32)
    csthi = np.concatenate([CT[:128], ST[:128], -ST[:128], CT[:128]], axis=1)
    cstlo8 = np.concatenate([
        np.concatenate([CT[128:], ST[128:]], axis=1),
        np.concatenate([-ST[128:], CT[128:]], axis=1)], axis=0)  # (8,256)

    w_v = np.ones(NV)
    w_v[1:NV - 1] = 2.0
    RC = (Cm[:, :NV] * w_v[None, :]).T.astype(np.float32)      # (67,128)
    RS = (-Sm[:, :NV] * w_v[None, :]).T.astype(np.float32)
    rcs2 = np.concatenate([RC, RS, RS, RC], axis=1)            # (67,512)

    return {
        "khi": np.ascontiguousarray(khi),
        "klo": np.ascontiguousarray(klo),
        "gt": gt,
        "gtv": gtv,
        "ident": np.eye(NV, dtype=np.float32),
        "csthi": np.ascontiguousarray(csthi.astype(np.float32)),
        "cstlo8": np.ascontiguousarray(cstlo8.astype(np.float32)),
        "rcs2": np.ascontiguousarray(rcs2.astype(np.float32)),
    }


# --------------------------------------------------------------------------
# device kernel
# --------------------------------------------------------------------------

def build_nc(n_chan=CPC, n_batch=B, gelu=True):
    act_fn = AF.Gelu if gelu else AF.Copy
    n_img = n_chan * n_batch
    nc = bacc.Bacc("TRN2", target_bir_lowering=False, debug=False,
                   enable_asserts=False)

    x_t = nc.dram_tensor("x", [n_img, N0, N0], F32R, kind="ExternalInput")
    khi_t = nc.dram_tensor("khi", [n_chan, 128, 2 * PH4], F32,
                           kind="ExternalInput")
    klo_t = nc.dram_tensor("klo", [n_chan, 4, 2 * PH4], F32,
                           kind="ExternalInput")
    gt_t = nc.dram_tensor("gt", [128, 2 * NP], F32R, kind="ExternalInput")
    gtv_t = nc.dram_tensor("gtv", [128, 3 * NV], F32R, kind="ExternalInput")
    ident_t = nc.dram_tensor("ident", [NV, NV], F32, kind="ExternalInput")
    csthi_t = nc.dram_tensor("csthi", [128, 512], F32R, kind="ExternalInput")
    cstlo8_t = nc.dram_tensor("cstlo8", [8, 256], F32R, kind="ExternalInput")
    rcs2_t = nc.dram_tensor("rcs2", [NV, 512], F32R, kind="ExternalInput")
    out_t = nc.dram_tensor("out", [n_img, 2 * N0, 2 * N0], F32,
                           kind="ExternalOutput")

    with tile.TileContext(nc) as tc:
        with (
            tc.tile_pool(name="consts", bufs=1) as cpool,
            tc.tile_pool(name="kdd", bufs=2) as kpool,
            tc.tile_pool(name="xin", bufs=4) as xpool,
            tc.tile_pool(name="r1", bufs=2) as r1pool,
            tc.tile_pool(name="ysb", bufs=2) as ypool,
            tc.tile_pool(name="fxt", bufs=2) as fxtpool,
            tc.tile_pool(name="fx", bufs=2) as fxpool,
            tc.tile_pool(name="t1", bufs=2) as t1pool,
            tc.tile_pool(name="osb", bufs=2) as opool,
            tc.tile_pool(name="ppa", bufs=1, space="PSUM") as ppa_pool,
            tc.tile_pool(name="ppy", bufs=1, space="PSUM") as ppy_pool,
            tc.tile_pool(name="pyt", bufs=1, space="PSUM") as pyt_pool,
            tc.tile_pool(name="pt1", bufs=2, space="PSUM") as pt1_pool,
            tc.tile_pool(name="ppd", bufs=2, space="PSUM") as ppd_pool,
        ):
            gt = cpool.tile([128, 2 * NP], F32R)
            nc.sync.dma_start(gt[:], gt_t[:])
            gtv = cpool.tile([128, 3 * NV], F32R)
            nc.sync.dma_start(gtv[:], gtv_t[:])
            ident = cpool.tile([NV, NV], F32)
            nc.sync.dma_start(ident[:], ident_t[:])
            csthi = cpool.tile([128, 512], F32R)
            nc.sync.dma_start(csthi[:], csthi_t[:])
            cstlo_r = cpool.tile([4, 256], F32R)
            nc.sync.dma_start(cstlo_r[:], cstlo8_t[0:4])
            cstlo_i = cpool.tile([4, 256], F32R)
            nc.sync.dma_start(cstlo_i[:], cstlo8_t[4:8])
            rcs2 = cpool.tile([NV, 512], F32R)
            nc.sync.dma_start(rcs2[:], rcs2_t[:])

            for ci in range(n_chan):
                khi = kpool.tile([128, 2 * PH4], F32, tag="khi")
                nc.sync.dma_start(khi[:], khi_t[ci])
                klo = kpool.tile([4, 2 * PH4], F32, tag="klo")
                nc.sync.dma_start(klo[:], klo_t[ci])

                for pr in range(n_batch // 2):
                    r1g = r1pool.tile([128, 2 * NU], F32R, tag="r1g")

                    # ---- stage A: r1 = x^T @ [Gr^T|Gi^T] per image ----
                    for li in range(2):
                        img = ci * n_batch + 2 * pr + li
                        x_tile = xpool.tile([N0, N0], F32R, tag="x")
                        nc.sync.dma_start(x_tile[:], x_t[img])
                        pA = ppa_pool.tile([128, NU], F32, tag="pA")
                        nc.tensor.matmul(pA[:], x_tile[:], gt[:],
                                         start=True, stop=True)
                        nc.scalar.activation(
                            r1g[:, li * NU:(li + 1) * NU], pA[:], AF.Copy)

                    # ---- stage B: Y^T = G-cols^T @ r1, both images ----
                    r1v = r1g[:].rearrange("p (i u) -> p i u", i=2)
                    r_cols = r1v[:, :, 0:NP]
                    i_cols = r1v[:, :, NP:2 * NP]
                    pYr = ppy_pool.tile([NV, 2 * NP], F32, tag="pYr")
                    pYi = ppy_pool.tile([NV, 2 * NP], F32, tag="pYi")
                    nc.tensor.matmul(pYr[:], gtv[:, NV:2 * NV], i_cols,
                                     start=True, stop=False)
                    nc.tensor.matmul(pYi[:], gtv[:, 2 * NV:3 * NV], r_cols,
                                     start=True, stop=False)
                    nc.tensor.matmul(pYr[:], gtv[:, 0:NV], r_cols,
                                     start=False, stop=True)
                    nc.tensor.matmul(pYi[:], gtv[:, 0:NV], i_cols,
                                     start=False, stop=True)

                    # evict pair Y to SBUF: y_sb[v, (i, ri, u)]
                    y_sb = ypool.tile([NV, 2 * NU], F32, tag="ysb")
                    yv = y_sb[:].rearrange("p (i r u) -> p i r u", i=2, r=2)
                    nc.scalar.activation(
                        yv[:, :, 0, :],
                        pYr[:].rearrange("p (i u) -> p i u", i=2), AF.Copy)
                    nc.scalar.activation(
                        yv[:, :, 1, :],
                        pYi[:].rearrange("p (i u) -> p i u", i=2), AF.Copy)

                    for li in range(2):
                        img = ci * n_batch + 2 * pr + li
                        base = li * NU

                        # ---- transposes: Y^T -> Y[u, v] (hi/lo) ----
                        yT = pyt_pool.tile([128, 4 * NV], F32, tag="yT")
                        nc.tensor.transpose(
                            yT[:, 0:NV], y_sb[:, base:base + 128], ident[:])
                        nc.tensor.transpose(
                            yT[:, NV:2 * NV],
                            y_sb[:, base + NP:base + NP + 128], ident[:])
                        nc.tensor.transpose(
                            yT[0:4, 2 * NV:3 * NV],
                            y_sb[:, base + 128:base + NP], ident[:])
                        nc.tensor.transpose(
                            yT[0:4, 3 * NV:4 * NV],
                            y_sb[:, base + NP + 128:base + 2 * NP], ident[:])

                        # ---- FX = K (.) Y (complex mult, DVE) ----
                        yrb = yT[:, 0:NV][:, None, :].broadcast_to(
                            [128, 4, NV])
                        yib = yT[:, NV:2 * NV][:, None, :].broadcast_to(
                            [128, 4, NV])
                        ta = fxtpool.tile([128, PH4], F32, tag="ta")
                        tb = fxtpool.tile([128, PH4], F32, tag="tb")
                        kr_v = khi[:, 0:PH4].rearrange("p (f v) -> p f v", f=4)
                        ki_v = khi[:, PH4:2 * PH4].rearrange(
                            "p (f v) -> p f v", f=4)
                        nc.vector.tensor_mul(
                            ta[:].rearrange("p (f v) -> p f v", f=4),
                            kr_v, yrb)
                        nc.vector.tensor_mul(
                            tb[:].rearrange("p (f v) -> p f v", f=4),
                            ki_v, yib)
                        fxr_hi = fxpool.tile([128, PH4], F32R, tag="fxr_hi")
                        nc.vector.tensor_sub(fxr_hi[:], ta[:], tb[:])
                        tc_ = fxtpool.tile([128, PH4], F32, tag="tc")
                        td = fxtpool.tile([128, PH4], F32, tag="td")
                        nc.vector.tensor_mul(
                            tc_[:].rearrange("p (f v) -> p f v", f=4),
                            kr_v, yib)
                        nc.vector.tensor_mul(
                            td[:].rearrange("p (f v) -> p f v", f=4),
                            ki_v, yrb)
                        fxi_hi = fxpool.tile([128, PH4], F32R, tag="fxi_hi")
                        nc.vector.tensor_add(fxi_hi[:], tc_[:], td[:])

                        # lo rows (u=128..131): fxlo8 = [[FXr_lo];[FXi_lo]]
                        yrlb = yT[0:4, 2 * NV:3 * NV][:, None, :].broadcast_to(
                            [4, 4, NV])
                        yilb = yT[0:4, 3 * NV:4 * NV][:, None, :].broadcast_to(
                            [4, 4, NV])
                        tal = fxtpool.tile([4, PH4], F32, tag="tal")
                        tbl = fxtpool.tile([4, PH4], F32, tag="tbl")
                        krl_v = klo[:, 0:PH4].rearrange("p (f v) -> p f v", f=4)
                        kil_v = klo[:, PH4:2 * PH4].rearrange(
                            "p (f v) -> p f v", f=4)
                        fxr_lo = fxpool.tile([4, PH4], F32R, tag="fxr_lo")
                        fxi_lo = fxpool.tile([4, PH4], F32R, tag="fxi_lo")
                        nc.vector.tensor_mul(
                            tal[:].rearrange("p (f v) -> p f v", f=4),
                            krl_v, yrlb)
                        nc.vector.tensor_mul(
                            tbl[:].rearrange("p (f v) -> p f v", f=4),
                            kil_v, yilb)
                        nc.vector.tensor_sub(fxr_lo[:], tal[:], tbl[:])
                        nc.vector.tensor_mul(
                            tal[:].rearrange("p (f v) -> p f v", f=4),
                            krl_v, yilb)
                        nc.vector.tensor_mul(
                            tbl[:].rearrange("p (f v) -> p f v", f=4),
                            kil_v, yrlb)
                        nc.vector.tensor_add(fxi_lo[:], tal[:], tbl[:])

                        # ---- stage C: T1^T[v, (x_r | x_i)] per phase ----
                        t1g = t1pool.tile([NV, 1024], F32R, tag="t1g")
                        for p in range(4):
                            pT1 = pt1_pool.tile([NV, 256], F32, tag="pT1")
                            sl = slice(p * NV, (p + 1) * NV)
                            nc.tensor.matmul(pT1[:], fxr_hi[:, sl],
                                             csthi[:, 0:256],
                                             start=True, stop=False)
                            nc.tensor.matmul(pT1[:], fxi_hi[:, sl],
                                             csthi[:, 256:512],
                                             start=False, stop=False)
                            nc.tensor.matmul(pT1[:], fxr_lo[:, sl],
                                             cstlo_r[:],
                                             start=False, stop=False)
                            nc.tensor.matmul(pT1[:], fxi_lo[:, sl],
                                             cstlo_i[:],
                                             start=False, stop=True)
                            dst = t1g[:, p * 256:(p + 1) * 256]
                            if p < 2:
                                nc.vector.tensor_copy(dst, pT1[:])
                            else:
                                nc.scalar.activation(dst, pT1[:], AF.Copy)

                        # ---- stage D + gelu + interleaved store ----
                        orows = out_t[img].rearrange("(x d) y -> d x y", d=2)
                        for half in range(2):
                            pD = ppd_pool.tile([128, 512], F32, tag="pD")
                            for k in range(2):
                                p = 2 * half + k
                                o = pD[:, k * 256:(k + 1) * 256]
                                nc.tensor.matmul(
                                    o, t1g[:, p * 256:p * 256 + 128],
                                    rcs2[:, 0:256], start=True, stop=False)
                                nc.tensor.matmul(
                                    o, t1g[:, p * 256 + 128:(p + 1) * 256],
                                    rcs2[:, 256:512], start=False, stop=True)
                            outt = opool.tile([128, 256], F32, tag="outt")
                            nc.scalar.activation(
                                outt[:].rearrange("p (v d) -> p d v", d=2),
                                pD[:].rearrange("p (d h v) -> p d h v",
                                                d=2, h=2)[:, :, 0, :],
                                act_fn)
                            nc.sync.dma_start(orows[half], outt[:])

    nc.compile()
    return nc


# --------------------------------------------------------------------------
# public entry point: full inputs in, full output out
# --------------------------------------------------------------------------

def kernel(x, weight, bias):
    global LAST_EXEC_NS
    x = np.ascontiguousarray(np.asarray(x, dtype=np.float32))
    consts = _host_constants(weight, bias)

    nc = build_nc()

    in_maps = []
    for core in range(NCORES):
        c0 = core * CPC
        xs = np.ascontiguousarray(
            x[:, c0:c0 + CPC].transpose(1, 0, 2, 3)).reshape(NIMG, N0, N0)
        in_maps.append({
            "x": xs,
            "khi": np.ascontiguousarray(consts["khi"][c0:c0 + CPC]),
            "klo": np.ascontiguousarray(consts["klo"][c0:c0 + CPC]),
            "gt": consts["gt"],
            "gtv": consts["gtv"],
            "ident": consts["ident"],
            "csthi": consts["csthi"],
            "cstlo8": consts["cstlo8"],
            "rcs2": consts["rcs2"],
        })

    trace = os.environ.get("KERNEL_TRACE", "0") == "1"
    tmpdir = os.environ.get("KERNEL_TMPDIR") or None
    res = run_bass_kernel_spmd(nc, in_maps, list(range(NCORES)), trace=trace,
                               tmpdir=tmpdir)
    LAST_EXEC_NS = res.exec_time_ns

    out = np.empty((B, C, 2 * N0, 2 * N0), dtype=np.float32)
    for core in range(NCORES):
        c0 = core * CPC
        o = res.results[core]["out"].reshape(CPC, B, 2 * N0, 2 * N0)
        out[:, c0:c0 + CPC] = o.transpose(1, 0, 2, 3)
    return out


# revision 11
# speedup vs baseline: 2.8048x; 1.1045x over previous
"""Converse2D-Up (FFT deconvolution upsampler) as a Bass/Tile kernel for TRN2.

Restructured for f32r matmuls (FP22 multiply, 1 cyc/row at N>=256 vs
fp32's 4 cyc/row + 2 HW passes).  Dataflow per image (validated in numpy
against the jax reference to rel-l2 ~1.1e-4):

  A: r1[y,u_ri] = x^T @ [Gr^T|Gi^T]           (stat=x, N=264)
  B: Y^T[v,u] per img-pair, const-stationary   (stat=G-cols, N=264)
  T: 4 PE transposes Y^T -> Y[u,v] hi/lo       (fp32, small)
  FX: complex mult K (.) Y on DVE              ([u,(p,v)] layout)
  C: T1^T[v,(x|x)] = FX-stat @ [C|S] consts    (N=256, lo rows merged
     into one K=8 matmul via [[FXr_lo];[FXi_lo]] stacking)
  D: out_p[x,(y|junk)] = T1-stat @ [RC|RS]     (N=256)
  gelu + phase interleave on ScalarE eviction, as before.

Sharding: 8 channels per core x 4 batch images; weight/bias spectra are
host-precomputed constants.
"""

import os

import numpy as np

import concourse.bass as bass
import concourse.mybir as mybir
import concourse.tile as tile
from concourse import bacc
from concourse.bass_utils import run_bass_kernel_spmd

F32 = mybir.dt.float32
F32R = mybir.dt.float32r
AF = mybir.ActivationFunctionType

SCALE = 2
PAD = 2
EPS = 1e-5
N0 = 128           # input spatial size
NP = N0 + 2 * PAD  # 132 padded
NU = NP * SCALE    # 264 upsampled
NV = NP // 2 + 1   # 67 unique spectral columns
B = 4
C = 64
NCORES = 8
CPC = C // NCORES  # 8 channels per core
NIMG = B * CPC     # 32 images per core
PH4 = 4 * NV       # 268

LAST_EXEC_NS = None  # set by kernel() when tracing is enabled


# --------------------------------------------------------------------------
# host-side constant precompute (weight/bias -> per-channel spectra)
# --------------------------------------------------------------------------

def _host_constants(weight, bias):
    w64 = np.asarray(weight, dtype=np.float64)
    b64 = np.asarray(bias, dtype=np.float64)

    k_h, k_w = w64.shape[-2:]
    otf = np.zeros((C, NU, NU), dtype=np.complex128)
    otf[:, :k_h, :k_w] = w64[0]
    otf = np.roll(otf, (-(k_h // 2), -(k_w // 2)), axis=(-2, -1))
    FB = np.fft.fftn(otf, axes=(-2, -1))                      # (C,264,264)

    biaseps = 1.0 / (1.0 + np.exp(-(b64.reshape(C) - 9.0))) + EPS
    be = biaseps[:, None, None]

    u = np.arange(NU)
    Dr = 1 + np.exp(-2j * np.pi * u / NU)
    D = Dr[:, None] * Dr[None, :]

    Gh = np.conj(FB) + be * D[None]
    FBG = FB * Gh

    def quadmean(A):
        return 0.25 * (A[:, :NP, :NP] + A[:, NP:, :NP]
                       + A[:, :NP, NP:] + A[:, NP:, NP:])

    M1 = quadmean(FBG)
    invW = quadmean(np.abs(FB) ** 2)
    M2 = M1 / (invW + be)
    H = (Gh - np.conj(FB) * np.tile(M2, (1, SCALE, SCALE))) / be

    hr = np.fft.ifft2(H, axes=(-2, -1)).real
    kdd = np.empty((C, 2, 2, NP, NV), dtype=np.complex128)
    for dx in range(2):
        for dy in range(2):
            kh = np.fft.fft2(hr[:, dx::2, dy::2], axes=(-2, -1))
            kdd[:, dx, dy] = kh[:, :, :NV]

    # K tiles in [u, (ri, p, v)] layout
    khi = np.empty((C, 128, 2 * PH4), dtype=np.float32)
    klo = np.empty((C, 4, 2 * PH4), dtype=np.float32)
    for dx in range(2):
        for dy in range(2):
            p = dx * 2 + dy
            kr = kdd[:, dx, dy].real
            ki = kdd[:, dx, dy].imag
            khi[:, :, p * NV:(p + 1) * NV] = kr[:, :128]
            khi[:, :, PH4 + p * NV:PH4 + (p + 1) * NV] = ki[:, :128]
            klo[:, :, p * NV:(p + 1) * NV] = kr[:, 128:]
            klo[:, :, PH4 + p * NV:PH4 + (p + 1) * NV] = ki[:, 128:]

    # forward matrix G = F132 @ P  (132x128 complex)
    P = np.zeros((NP, N0))
    for m in range(NP):
        P[m, (m - PAD) % N0] = 1.0
    F132 = np.exp(-2j * np.pi * np.outer(np.arange(NP), np.arange(NP)) / NP)
    G = F132 @ P
    gt = np.concatenate([G.real.T, G.imag.T], axis=1).astype(np.float32)

    # stage-B stationaries: [Gr_v | -Gi_v | Gi_v]  (128, 201)
    gtv = np.concatenate([gt[:, 0:NV], -gt[:, NP:NP + NV],
                          gt[:, NP:NP + NV]], axis=1).astype(np.float32)

    # inverse rows i in [2,130) of iF132/132
    Ai = np.exp(2j * np.pi * np.outer(np.arange(2, 130), np.arange(NP)) / NP) / NP
    Cm, Sm = Ai.real, Ai.imag
    CT = Cm.T.astype(np.float32)                               # (132,128)
    ST = Sm.T.astype(np.float32)
    csthi = np.concatenate([CT[:128], ST[:128], -ST[:128], CT[:128]], axis=1)
    cstlo8 = np.concatenate([
        np.concatenate([CT[128:], ST[128:]], axis=1),
        np.concatenate([-ST[128:], CT[128:]], axis=1)], axis=0)  # (8,256)

    w_v = np.ones(NV)
    w_v[1:NV - 1] = 2.0
    RC = (Cm[:, :NV] * w_v[None, :]).T.astype(np.float32)      # (67,128)
    RS = (-Sm[:, :NV] * w_v[None, :]).T.astype(np.float32)
    rcs2 = np.concatenate([RC, RS, RS, RC], axis=1)            # (67,512)

    # swapped copies [Ki | Kr] so both FX product pairs share one layout
    khi2 = np.concatenate([khi[:, :, PH4:], khi[:, :, :PH4]], axis=2)
    klo2 = np.concatenate([klo[:, :, PH4:], klo[:, :, :PH4]], axis=2)

    return {
        "khi": np.ascontiguousarray(khi),
        "klo": np.ascontiguousarray(klo),
        "khi2": np.ascontiguousarray(khi2),
        "klo2": np.ascontiguousarray(klo2),
        "gt": gt,
        "gtv": gtv,
        "ident": np.eye(NV, dtype=np.float32),
        "csthi": np.ascontiguousarray(csthi.astype(np.float32)),
        "cstlo8": np.ascontiguousarray(cstlo8.astype(np.float32)),
        "rcs2": np.ascontiguousarray(rcs2.astype(np.float32)),
    }


# --------------------------------------------------------------------------
# device kernel
# --------------------------------------------------------------------------

def build_nc(n_chan=CPC, n_batch=B, gelu=True):
    act_fn = AF.Gelu if gelu else AF.Copy
    n_img = n_chan * n_batch
    nc = bacc.Bacc("TRN2", target_bir_lowering=False, debug=False,
                   enable_asserts=False)

    x_t = nc.dram_tensor("x", [n_img, N0, N0], F32R, kind="ExternalInput")
    khi_t = nc.dram_tensor("khi", [n_chan, 128, 2 * PH4], F32,
                           kind="ExternalInput")
    klo_t = nc.dram_tensor("klo", [n_chan, 4, 2 * PH4], F32,
                           kind="ExternalInput")
    khi2_t = nc.dram_tensor("khi2", [n_chan, 128, 2 * PH4], F32,
                            kind="ExternalInput")
    klo2_t = nc.dram_tensor("klo2", [n_chan, 4, 2 * PH4], F32,
                            kind="ExternalInput")
    gt_t = nc.dram_tensor("gt", [128, 2 * NP], F32R, kind="ExternalInput")
    gtv_t = nc.dram_tensor("gtv", [128, 3 * NV], F32R, kind="ExternalInput")
    ident_t = nc.dram_tensor("ident", [NV, NV], F32, kind="ExternalInput")
    csthi_t = nc.dram_tensor("csthi", [128, 512], F32R, kind="ExternalInput")
    cstlo8_t = nc.dram_tensor("cstlo8", [8, 256], F32R, kind="ExternalInput")
    rcs2_t = nc.dram_tensor("rcs2", [NV, 512], F32R, kind="ExternalInput")
    out_t = nc.dram_tensor("out", [n_img, 2 * N0, 2 * N0], F32,
                           kind="ExternalOutput")

    with tile.TileContext(nc) as tc:
        with (
            tc.tile_pool(name="consts", bufs=1) as cpool,
            tc.tile_pool(name="kdd", bufs=2) as kpool,
            tc.tile_pool(name="xin", bufs=4) as xpool,
            tc.tile_pool(name="r1", bufs=2) as r1pool,
            tc.tile_pool(name="ysb", bufs=2) as ypool,
            tc.tile_pool(name="fxt", bufs=2) as fxtpool,
            tc.tile_pool(name="fx", bufs=2) as fxpool,
            tc.tile_pool(name="t1", bufs=2) as t1pool,
            tc.tile_pool(name="osb", bufs=2) as opool,
            tc.tile_pool(name="ppa", bufs=1, space="PSUM") as ppa_pool,
            tc.tile_pool(name="ppy", bufs=1, space="PSUM") as ppy_pool,
            tc.tile_pool(name="pyt", bufs=1, space="PSUM") as pyt_pool,
            tc.tile_pool(name="pt1", bufs=2, space="PSUM") as pt1_pool,
            tc.tile_pool(name="ppd", bufs=2, space="PSUM") as ppd_pool,
        ):
            gt = cpool.tile([128, 2 * NP], F32R)
            nc.sync.dma_start(gt[:], gt_t[:])
            gtv = cpool.tile([128, 3 * NV], F32R)
            nc.sync.dma_start(gtv[:], gtv_t[:])
            ident = cpool.tile([NV, NV], F32)
            nc.sync.dma_start(ident[:], ident_t[:])
            csthi = cpool.tile([128, 512], F32R)
            nc.sync.dma_start(csthi[:], csthi_t[:])
            cstlo_r = cpool.tile([4, 256], F32R)
            nc.sync.dma_start(cstlo_r[:], cstlo8_t[0:4])
            cstlo_i = cpool.tile([4, 256], F32R)
            nc.sync.dma_start(cstlo_i[:], cstlo8_t[4:8])
            rcs2 = cpool.tile([NV, 512], F32R)
            nc.sync.dma_start(rcs2[:], rcs2_t[:])

            for ci in range(n_chan):
                khi = kpool.tile([128, 2 * PH4], F32, tag="khi")
                nc.sync.dma_start(khi[:], khi_t[ci])
                klo = kpool.tile([4, 2 * PH4], F32, tag="klo")
                nc.sync.dma_start(klo[:], klo_t[ci])
                khi2 = kpool.tile([128, 2 * PH4], F32, tag="khi2")
                nc.sync.dma_start(khi2[:], khi2_t[ci])
                klo2 = kpool.tile([4, 2 * PH4], F32, tag="klo2")
                nc.sync.dma_start(klo2[:], klo2_t[ci])

                for pr in range(n_batch // 2):
                    r1g = r1pool.tile([128, 2 * NU], F32R, tag="r1g")

                    # ---- stage A: r1 = x^T @ [Gr^T|Gi^T] per image ----
                    for li in range(2):
                        img = ci * n_batch + 2 * pr + li
                        x_tile = xpool.tile([N0, N0], F32R, tag="x")
                        nc.sync.dma_start(x_tile[:], x_t[img])
                        pA = ppa_pool.tile([128, NU], F32, tag="pA")
                        nc.tensor.matmul(pA[:], x_tile[:], gt[:],
                                         start=True, stop=True)
                        nc.scalar.activation(
                            r1g[:, li * NU:(li + 1) * NU], pA[:], AF.Copy)

                    # ---- stage B: Y^T = G-cols^T @ r1, both images ----
                    r1v = r1g[:].rearrange("p (i u) -> p i u", i=2)
                    r_cols = r1v[:, :, 0:NP]
                    i_cols = r1v[:, :, NP:2 * NP]
                    pYr = ppy_pool.tile([NV, 2 * NP], F32, tag="pYr")
                    pYi = ppy_pool.tile([NV, 2 * NP], F32, tag="pYi")
                    nc.tensor.matmul(pYr[:], gtv[:, NV:2 * NV], i_cols,
                                     start=True, stop=False)
                    nc.tensor.matmul(pYi[:], gtv[:, 2 * NV:3 * NV], r_cols,
                                     start=True, stop=False)
                    nc.tensor.matmul(pYr[:], gtv[:, 0:NV], r_cols,
                                     start=False, stop=True)
                    nc.tensor.matmul(pYi[:], gtv[:, 0:NV], i_cols,
                                     start=False, stop=True)

                    # evict pair Y to SBUF: y_sb[v, (i, ri, u)]
                    y_sb = ypool.tile([NV, 2 * NU], F32, tag="ysb")
                    yv = y_sb[:].rearrange("p (i r u) -> p i r u", i=2, r=2)
                    nc.scalar.activation(
                        yv[:, :, 0, :],
                        pYr[:].rearrange("p (i u) -> p i u", i=2), AF.Copy)
                    nc.scalar.activation(
                        yv[:, :, 1, :],
                        pYi[:].rearrange("p (i u) -> p i u", i=2), AF.Copy)

                    for li in range(2):
                        img = ci * n_batch + 2 * pr + li
                        base = li * NU

                        # ---- transposes: Y^T -> Y[u, v] (hi/lo) ----
                        yT = pyt_pool.tile([128, 4 * NV], F32, tag="yT")
                        nc.tensor.transpose(
                            yT[:, 0:NV], y_sb[:, base:base + 128], ident[:])
                        nc.tensor.transpose(
                            yT[:, NV:2 * NV],
                            y_sb[:, base + NP:base + NP + 128], ident[:])
                        nc.tensor.transpose(
                            yT[0:4, 2 * NV:3 * NV],
                            y_sb[:, base + 128:base + NP], ident[:])
                        nc.tensor.transpose(
                            yT[0:4, 3 * NV:4 * NV],
                            y_sb[:, base + NP + 128:base + 2 * NP], ident[:])

                        # ---- FX = K (.) Y (complex mult, DVE) ----
                        # yb = [Yr | Yi] broadcast over phases: [p, 2, 4, 67]
                        yb = (yT[:, 0:2 * NV]
                              .rearrange("p (r v) -> p r v", r=2)
                              [:, :, None, :].broadcast_to([128, 2, 4, NV]))
                        ta = fxtpool.tile([128, 2 * PH4], F32, tag="ta")
                        tb = fxtpool.tile([128, 2 * PH4], F32, tag="tb")
                        # ta = [Kr*Yr | Ki*Yi], tb = [Ki*Yr | Kr*Yi]
                        nc.vector.tensor_mul(
                            ta[:].rearrange("p (r f v) -> p r f v", r=2, f=4),
                            khi[:].rearrange("p (r f v) -> p r f v",
                                             r=2, f=4), yb)
                        nc.vector.tensor_mul(
                            tb[:].rearrange("p (r f v) -> p r f v", r=2, f=4),
                            khi2[:].rearrange("p (r f v) -> p r f v",
                                              r=2, f=4), yb)
                        fxr_hi = fxpool.tile([128, PH4], F32R, tag="fxr_hi")
                        nc.vector.tensor_sub(fxr_hi[:], ta[:, 0:PH4],
                                             ta[:, PH4:2 * PH4])
                        fxi_hi = fxpool.tile([128, PH4], F32R, tag="fxi_hi")
                        nc.vector.tensor_add(fxi_hi[:], tb[:, 0:PH4],
                                             tb[:, PH4:2 * PH4])

                        # lo rows (u=128..131)
                        ylb = (yT[0:4, 2 * NV:4 * NV]
                               .rearrange("p (r v) -> p r v", r=2)
                               [:, :, None, :].broadcast_to([4, 2, 4, NV]))
                        tal = fxtpool.tile([4, 2 * PH4], F32, tag="tal")
                        tbl = fxtpool.tile([4, 2 * PH4], F32, tag="tbl")
                        nc.vector.tensor_mul(
                            tal[:].rearrange("p (r f v) -> p r f v", r=2, f=4),
                            klo[:].rearrange("p (r f v) -> p r f v",
                                             r=2, f=4), ylb)
                        nc.vector.tensor_mul(
                            tbl[:].rearrange("p (r f v) -> p r f v", r=2, f=4),
                            klo2[:].rearrange("p (r f v) -> p r f v",
                                              r=2, f=4), ylb)
                        fxr_lo = fxpool.tile([4, PH4], F32R, tag="fxr_lo")
                        fxi_lo = fxpool.tile([4, PH4], F32R, tag="fxi_lo")
                        nc.vector.tensor_sub(fxr_lo[:], tal[:, 0:PH4],
                                             tal[:, PH4:2 * PH4])
                        nc.vector.tensor_add(fxi_lo[:], tbl[:, 0:PH4],
                                             tbl[:, PH4:2 * PH4])

                        # ---- stage C: T1^T[v, (x_r | x_i)], 2 phases/bank ----
                        t1g = t1pool.tile([NV, 1024], F32R, tag="t1g")
                        for ph in range(2):
                            pT1 = pt1_pool.tile([NV, 512], F32, tag="pT1")
                            for k in range(2):
                                p = 2 * ph + k
                                o = pT1[:, k * 256:(k + 1) * 256]
                                sl = slice(p * NV, (p + 1) * NV)
                                nc.tensor.matmul(o, fxr_hi[:, sl],
                                                 csthi[:, 0:256],
                                                 start=True, stop=False)
                                nc.tensor.matmul(o, fxi_hi[:, sl],
                                                 csthi[:, 256:512],
                                                 start=False, stop=False)
                                nc.tensor.matmul(o, fxr_lo[:, sl],
                                                 cstlo_r[:],
                                                 start=False, stop=False)
                                nc.tensor.matmul(o, fxi_lo[:, sl],
                                                 cstlo_i[:],
                                                 start=False, stop=True)
                            dst = t1g[:, ph * 512:(ph + 1) * 512]
                            if ph == 0:
                                nc.vector.tensor_copy(dst, pT1[:])
                            else:
                                nc.scalar.activation(dst, pT1[:], AF.Copy)

                        # ---- stage D + gelu + interleaved store ----
                        orows = out_t[img].rearrange("(x d) y -> d x y", d=2)
                        for half in range(2):
                            pD = ppd_pool.tile([128, 512], F32, tag="pD")
                            for k in range(2):
                                p = 2 * half + k
                                o = pD[:, k * 256:(k + 1) * 256]
                                nc.tensor.matmul(
                                    o, t1g[:, p * 256:p * 256 + 128],
                                    rcs2[:, 0:256], start=True, stop=False)
                                nc.tensor.matmul(
                                    o, t1g[:, p * 256 + 128:(p + 1) * 256],
                                    rcs2[:, 256:512], start=False, stop=True)
                            outt = opool.tile([128, 256], F32, tag="outt")
                            nc.scalar.activation(
                                outt[:].rearrange("p (v d) -> p d v", d=2),
                                pD[:].rearrange("p (d h v) -> p d h v",
                                                d=2, h=2)[:, :, 0, :],
                                act_fn)
                            nc.sync.dma_start(orows[half], outt[:])

    nc.compile()
    return nc


# --------------------------------------------------------------------------
# public entry point: full inputs in, full output out
# --------------------------------------------------------------------------

def kernel(x, weight, bias):
    global LAST_EXEC_NS
    x = np.ascontiguousarray(np.asarray(x, dtype=np.float32))
    consts = _host_constants(weight, bias)

    nc = build_nc()

    in_maps = []
    for core in range(NCORES):
        c0 = core * CPC
        xs = np.ascontiguousarray(
            x[:, c0:c0 + CPC].transpose(1, 0, 2, 3)).reshape(NIMG, N0, N0)
        in_maps.append({
            "x": xs,
            "khi": np.ascontiguousarray(consts["khi"][c0:c0 + CPC]),
            "klo": np.ascontiguousarray(consts["klo"][c0:c0 + CPC]),
            "khi2": np.ascontiguousarray(consts["khi2"][c0:c0 + CPC]),
            "klo2": np.ascontiguousarray(consts["klo2"][c0:c0 + CPC]),
            "gt": consts["gt"],
            "gtv": consts["gtv"],
            "ident": consts["ident"],
            "csthi": consts["csthi"],
            "cstlo8": consts["cstlo8"],
            "rcs2": consts["rcs2"],
        })

    trace = os.environ.get("KERNEL_TRACE", "0") == "1"
    tmpdir = os.environ.get("KERNEL_TMPDIR") or None
    res = run_bass_kernel_spmd(nc, in_maps, list(range(NCORES)), trace=trace,
                               tmpdir=tmpdir)
    LAST_EXEC_NS = res.exec_time_ns

    out = np.empty((B, C, 2 * N0, 2 * N0), dtype=np.float32)
    for core in range(NCORES):
        c0 = core * CPC
        o = res.results[core]["out"].reshape(CPC, B, 2 * N0, 2 * N0)
        out[:, c0:c0 + CPC] = o.transpose(1, 0, 2, 3)
    return out


# revision 14
# speedup vs baseline: 3.0708x; 1.0949x over previous
"""Converse2D-Up (FFT deconvolution upsampler) as a Bass/Tile kernel for TRN2.

Restructured for f32r matmuls (FP22 multiply, 1 cyc/row at N>=256 vs
fp32's 4 cyc/row + 2 HW passes).  Dataflow per image (validated in numpy
against the jax reference to rel-l2 ~1.1e-4):

  A: r1[y,u_ri] = x^T @ [Gr^T|Gi^T]           (stat=x, N=264)
  B: Y^T[v,u] per img-pair, const-stationary   (stat=G-cols, N=264)
  T: 4 PE transposes Y^T -> Y[u,v] hi/lo       (fp32, small)
  FX: complex mult K (.) Y on DVE              ([u,(p,v)] layout)
  C: T1^T[v,(x|x)] = FX-stat @ [C|S] consts    (N=256, lo rows merged
     into one K=8 matmul via [[FXr_lo];[FXi_lo]] stacking)
  D: out_p[x,(y|junk)] = T1-stat @ [RC|RS]     (N=256)
  gelu + phase interleave on ScalarE eviction, as before.

Sharding: 8 channels per core x 4 batch images; weight/bias spectra are
host-precomputed constants.
"""

import os

import numpy as np

import concourse.bass as bass
import concourse.mybir as mybir
import concourse.tile as tile
from concourse import bacc
from concourse.bass_utils import run_bass_kernel_spmd

F32 = mybir.dt.float32
F32R = mybir.dt.float32r
AF = mybir.ActivationFunctionType

SCALE = 2
PAD = 2
EPS = 1e-5
N0 = 128           # input spatial size
NP = N0 + 2 * PAD  # 132 padded
NU = NP * SCALE    # 264 upsampled
NV = NP // 2 + 1   # 67 unique spectral columns
B = 4
C = 64
NCORES = 8
CPC = C // NCORES  # 8 channels per core
NIMG = B * CPC     # 32 images per core
PH4 = 4 * NV       # 268

LAST_EXEC_NS = None  # set by kernel() when tracing is enabled


# --------------------------------------------------------------------------
# host-side constant precompute (weight/bias -> per-channel spectra)
# --------------------------------------------------------------------------

def _host_constants(weight, bias):
    w64 = np.asarray(weight, dtype=np.float64)
    b64 = np.asarray(bias, dtype=np.float64)

    k_h, k_w = w64.shape[-2:]
    otf = np.zeros((C, NU, NU), dtype=np.complex128)
    otf[:, :k_h, :k_w] = w64[0]
    otf = np.roll(otf, (-(k_h // 2), -(k_w // 2)), axis=(-2, -1))
    FB = np.fft.fftn(otf, axes=(-2, -1))                      # (C,264,264)

    biaseps = 1.0 / (1.0 + np.exp(-(b64.reshape(C) - 9.0))) + EPS
    be = biaseps[:, None, None]

    u = np.arange(NU)
    Dr = 1 + np.exp(-2j * np.pi * u / NU)
    D = Dr[:, None] * Dr[None, :]

    Gh = np.conj(FB) + be * D[None]
    FBG = FB * Gh

    def quadmean(A):
        return 0.25 * (A[:, :NP, :NP] + A[:, NP:, :NP]
                       + A[:, :NP, NP:] + A[:, NP:, NP:])

    M1 = quadmean(FBG)
    invW = quadmean(np.abs(FB) ** 2)
    M2 = M1 / (invW + be)
    H = (Gh - np.conj(FB) * np.tile(M2, (1, SCALE, SCALE))) / be

    hr = np.fft.ifft2(H, axes=(-2, -1)).real
    kdd = np.empty((C, 2, 2, NP, NV), dtype=np.complex128)
    for dx in range(2):
        for dy in range(2):
            kh = np.fft.fft2(hr[:, dx::2, dy::2], axes=(-2, -1))
            kdd[:, dx, dy] = kh[:, :, :NV]

    # K tiles in [u, (ri, p, v)] layout
    khi = np.empty((C, 128, 2 * PH4), dtype=np.float32)
    klo = np.empty((C, 4, 2 * PH4), dtype=np.float32)
    for dx in range(2):
        for dy in range(2):
            p = dx * 2 + dy
            kr = kdd[:, dx, dy].real
            ki = kdd[:, dx, dy].imag
            khi[:, :, p * NV:(p + 1) * NV] = kr[:, :128]
            khi[:, :, PH4 + p * NV:PH4 + (p + 1) * NV] = ki[:, :128]
            klo[:, :, p * NV:(p + 1) * NV] = kr[:, 128:]
            klo[:, :, PH4 + p * NV:PH4 + (p + 1) * NV] = ki[:, 128:]

    # forward matrix G = F132 @ P  (132x128 complex)
    P = np.zeros((NP, N0))
    for m in range(NP):
        P[m, (m - PAD) % N0] = 1.0
    F132 = np.exp(-2j * np.pi * np.outer(np.arange(NP), np.arange(NP)) / NP)
    G = F132 @ P
    gt = np.concatenate([G.real.T, G.imag.T], axis=1).astype(np.float32)

    # stage-B movings: [Gr | Gi | -Gi | Gr]  (128, 268)
    gbv = np.concatenate([gt[:, 0:NV], gt[:, NP:NP + NV],
                          -gt[:, NP:NP + NV], gt[:, 0:NV]],
                         axis=1).astype(np.float32)

    # inverse rows i in [2,130) of iF132/132
    Ai = np.exp(2j * np.pi * np.outer(np.arange(2, 130), np.arange(NP)) / NP) / NP
    Cm, Sm = Ai.real, Ai.imag
    CT = Cm.T.astype(np.float32)                               # (132,128)
    ST = Sm.T.astype(np.float32)
    csthi = np.concatenate([CT[:128], ST[:128], -ST[:128], CT[:128]], axis=1)
    cstlo8 = np.concatenate([
        np.concatenate([CT[128:], ST[128:]], axis=1),
        np.concatenate([-ST[128:], CT[128:]], axis=1)], axis=0)  # (8,256)

    w_v = np.ones(NV)
    w_v[1:NV - 1] = 2.0
    RC = (Cm[:, :NV] * w_v[None, :]).T.astype(np.float32)      # (67,128)
    RS = (-Sm[:, :NV] * w_v[None, :]).T.astype(np.float32)
    rcs2 = np.concatenate([RC, RS, RS, RC], axis=1)            # (67,512)

    # swapped copies [Ki | Kr] so both FX product pairs share one layout
    khi2 = np.concatenate([khi[:, :, PH4:], khi[:, :, :PH4]], axis=2)
    klo2 = np.concatenate([klo[:, :, PH4:], klo[:, :, :PH4]], axis=2)

    return {
        "khi": np.ascontiguousarray(khi),
        "klo": np.ascontiguousarray(klo),
        "khi2": np.ascontiguousarray(khi2),
        "klo2": np.ascontiguousarray(klo2),
        "gt": gt,
        "gbv": gbv,
        "csthi": np.ascontiguousarray(csthi.astype(np.float32)),
        "cstlo8": np.ascontiguousarray(cstlo8.astype(np.float32)),
        "rcs2": np.ascontiguousarray(rcs2.astype(np.float32)),
    }


# --------------------------------------------------------------------------
# device kernel
# --------------------------------------------------------------------------

def build_nc(n_chan=CPC, n_batch=B, gelu=True):
    act_fn = AF.Gelu if gelu else AF.Copy
    n_img = n_chan * n_batch
    nc = bacc.Bacc("TRN2", target_bir_lowering=False, debug=False,
                   enable_asserts=False)

    x_t = nc.dram_tensor("x", [n_img, N0, N0], F32R, kind="ExternalInput")
    khi_t = nc.dram_tensor("khi", [n_chan, 128, 2 * PH4], F32,
                           kind="ExternalInput")
    klo_t = nc.dram_tensor("klo", [n_chan, 4, 2 * PH4], F32,
                           kind="ExternalInput")
    khi2_t = nc.dram_tensor("khi2", [n_chan, 128, 2 * PH4], F32,
                            kind="ExternalInput")
    klo2_t = nc.dram_tensor("klo2", [n_chan, 4, 2 * PH4], F32,
                            kind="ExternalInput")
    gt_t = nc.dram_tensor("gt", [128, 2 * NP], F32R, kind="ExternalInput")
    gbv_t = nc.dram_tensor("gbv", [128, 4 * NV], F32R, kind="ExternalInput")
    csthi_t = nc.dram_tensor("csthi", [128, 512], F32R, kind="ExternalInput")
    cstlo8_t = nc.dram_tensor("cstlo8", [8, 256], F32R, kind="ExternalInput")
    rcs2_t = nc.dram_tensor("rcs2", [NV, 512], F32R, kind="ExternalInput")
    out_t = nc.dram_tensor("out", [n_img, 2 * N0, 2 * N0], F32,
                           kind="ExternalOutput")

    with tile.TileContext(nc) as tc:
        with (
            tc.tile_pool(name="consts", bufs=1) as cpool,
            tc.tile_pool(name="kdd", bufs=2) as kpool,
            tc.tile_pool(name="xin", bufs=4) as xpool,
            tc.tile_pool(name="r1", bufs=2) as r1pool,
            tc.tile_pool(name="fxt", bufs=2) as fxtpool,
            tc.tile_pool(name="fx", bufs=2) as fxpool,
            tc.tile_pool(name="t1", bufs=2) as t1pool,
            tc.tile_pool(name="osb", bufs=2) as opool,
            tc.tile_pool(name="ppa", bufs=1, space="PSUM") as ppa_pool,
            tc.tile_pool(name="pyt", bufs=2, space="PSUM") as pyt_pool,
            tc.tile_pool(name="pylo", bufs=1, space="PSUM") as pylo_pool,
            tc.tile_pool(name="pt1", bufs=2, space="PSUM") as pt1_pool,
            tc.tile_pool(name="ppd", bufs=2, space="PSUM") as ppd_pool,
        ):
            gt = cpool.tile([128, 2 * NP], F32R)
            nc.sync.dma_start(gt[:], gt_t[:])
            gbv = cpool.tile([128, 4 * NV], F32R)
            nc.sync.dma_start(gbv[:], gbv_t[:])
            csthi = cpool.tile([128, 512], F32R)
            nc.sync.dma_start(csthi[:], csthi_t[:])
            cstlo_r = cpool.tile([4, 256], F32R)
            nc.sync.dma_start(cstlo_r[:], cstlo8_t[0:4])
            cstlo_i = cpool.tile([4, 256], F32R)
            nc.sync.dma_start(cstlo_i[:], cstlo8_t[4:8])
            rcs2 = cpool.tile([NV, 512], F32R)
            nc.sync.dma_start(rcs2[:], rcs2_t[:])

            for ci in range(n_chan):
                khi = kpool.tile([128, 2 * PH4], F32, tag="khi")
                nc.sync.dma_start(khi[:], khi_t[ci])
                klo = kpool.tile([4, 2 * PH4], F32, tag="klo")
                nc.sync.dma_start(klo[:], klo_t[ci])
                khi2 = kpool.tile([128, 2 * PH4], F32, tag="khi2")
                nc.sync.dma_start(khi2[:], khi2_t[ci])
                klo2 = kpool.tile([4, 2 * PH4], F32, tag="klo2")
                nc.sync.dma_start(klo2[:], klo2_t[ci])

                for pr in range(n_batch // 2):
                    r1g = r1pool.tile([128, 2 * NU], F32R, tag="r1g")

                    # ---- stage A: r1 = x^T @ [Gr^T|Gi^T] per image ----
                    for li in range(2):
                        img = ci * n_batch + 2 * pr + li
                        x_tile = xpool.tile([N0, N0], F32R, tag="x")
                        nc.sync.dma_start(x_tile[:], x_t[img])
                        pA = ppa_pool.tile([128, NU], F32, tag="pA")
                        nc.tensor.matmul(pA[:], x_tile[:], gt[:],
                                         start=True, stop=True)
                        nc.scalar.activation(
                            r1g[:, li * NU:(li + 1) * NU], pA[:], AF.Copy)

                    for li in range(2):
                        img = ci * n_batch + 2 * pr + li
                        base = li * NU

                        # ---- stage B (direct): Y[u, (r v | i v)] ----
                        yT = pyt_pool.tile([128, 2 * NV], F32, tag="yT")
                        ylo = pylo_pool.tile([4, 2 * NV], F32, tag="ylo")
                        nc.tensor.matmul(yT[:], r1g[:, base:base + 128],
                                         gbv[:, 0:2 * NV],
                                         start=True, stop=False)
                        nc.tensor.matmul(yT[:], r1g[:, base + NP:base + NP + 128],
                                         gbv[:, 2 * NV:4 * NV],
                                         start=False, stop=True)
                        nc.tensor.matmul(ylo[:], r1g[:, base + 128:base + NP],
                                         gbv[:, 0:2 * NV],
                                         start=True, stop=False)
                        nc.tensor.matmul(ylo[:], r1g[:, base + NP + 128:base + 2 * NP],
                                         gbv[:, 2 * NV:4 * NV],
                                         start=False, stop=True)

                        # ---- FX = K (.) Y (complex mult, DVE) ----
                        # yb = [Yr | Yi] broadcast over phases: [p, 2, 4, 67]
                        yb = (yT[:, 0:2 * NV]
                              .rearrange("p (r v) -> p r v", r=2)
                              [:, :, None, :].broadcast_to([128, 2, 4, NV]))
                        ta = fxtpool.tile([128, 2 * PH4], F32, tag="ta")
                        tb = fxtpool.tile([128, 2 * PH4], F32, tag="tb")
                        # ta = [Kr*Yr | Ki*Yi], tb = [Ki*Yr | Kr*Yi]
                        nc.vector.tensor_mul(
                            ta[:].rearrange("p (r f v) -> p r f v", r=2, f=4),
                            khi[:].rearrange("p (r f v) -> p r f v",
                                             r=2, f=4), yb)
                        nc.vector.tensor_mul(
                            tb[:].rearrange("p (r f v) -> p r f v", r=2, f=4),
                            khi2[:].rearrange("p (r f v) -> p r f v",
                                              r=2, f=4), yb)
                        fxr_hi = fxpool.tile([128, PH4], F32R, tag="fxr_hi")
                        nc.vector.tensor_sub(fxr_hi[:], ta[:, 0:PH4],
                                             ta[:, PH4:2 * PH4])
                        fxi_hi = fxpool.tile([128, PH4], F32R, tag="fxi_hi")
                        nc.vector.tensor_add(fxi_hi[:], tb[:, 0:PH4],
                                             tb[:, PH4:2 * PH4])

                        # lo rows (u=128..131)
                        ylb = (ylo[:]
                               .rearrange("p (r v) -> p r v", r=2)
                               [:, :, None, :].broadcast_to([4, 2, 4, NV]))
                        tal = fxtpool.tile([4, 2 * PH4], F32, tag="tal")
                        tbl = fxtpool.tile([4, 2 * PH4], F32, tag="tbl")
                        nc.vector.tensor_mul(
                            tal[:].rearrange("p (r f v) -> p r f v", r=2, f=4),
                            klo[:].rearrange("p (r f v) -> p r f v",
                                             r=2, f=4), ylb)
                        nc.vector.tensor_mul(
                            tbl[:].rearrange("p (r f v) -> p r f v", r=2, f=4),
                            klo2[:].rearrange("p (r f v) -> p r f v",
                                              r=2, f=4), ylb)
                        fxr_lo = fxpool.tile([4, PH4], F32R, tag="fxr_lo")
                        fxi_lo = fxpool.tile([4, PH4], F32R, tag="fxi_lo")
                        nc.vector.tensor_sub(fxr_lo[:], tal[:, 0:PH4],
                                             tal[:, PH4:2 * PH4])
                        nc.vector.tensor_add(fxi_lo[:], tbl[:, 0:PH4],
                                             tbl[:, PH4:2 * PH4])

                        # ---- stage C: T1^T[v, (x_r | x_i)], 2 phases/bank ----
                        t1g = t1pool.tile([NV, 1024], F32R, tag="t1g")
                        for ph in range(2):
                            pT1 = pt1_pool.tile([NV, 512], F32, tag="pT1")
                            for k in range(2):
                                p = 2 * ph + k
                                o = pT1[:, k * 256:(k + 1) * 256]
                                sl = slice(p * NV, (p + 1) * NV)
                                nc.tensor.matmul(o, fxr_hi[:, sl],
                                                 csthi[:, 0:256],
                                                 start=True, stop=False)
                                nc.tensor.matmul(o, fxi_hi[:, sl],
                                                 csthi[:, 256:512],
                                                 start=False, stop=False)
                                nc.tensor.matmul(o, fxr_lo[:, sl],
                                                 cstlo_r[:],
                                                 start=False, stop=False)
                                nc.tensor.matmul(o, fxi_lo[:, sl],
                                                 cstlo_i[:],
                                                 start=False, stop=True)
                            dst = t1g[:, ph * 512:(ph + 1) * 512]
                            if ph == 0:
                                nc.vector.tensor_copy(dst, pT1[:])
                            else:
                                nc.scalar.activation(dst, pT1[:], AF.Copy)

                        # ---- stage D + gelu + interleaved store ----
                        orows = out_t[img].rearrange("(x d) y -> d x y", d=2)
                        for half in range(2):
                            pD = ppd_pool.tile([128, 512], F32, tag="pD")
                            for k in range(2):
                                p = 2 * half + k
                                o = pD[:, k * 256:(k + 1) * 256]
                                nc.tensor.matmul(
                                    o, t1g[:, p * 256:p * 256 + 128],
                                    rcs2[:, 0:256], start=True, stop=False)
                                nc.tensor.matmul(
                                    o, t1g[:, p * 256 + 128:(p + 1) * 256],
                                    rcs2[:, 256:512], start=False, stop=True)
                            outt = opool.tile([128, 256], F32, tag="outt")
                            nc.scalar.activation(
                                outt[:].rearrange("p (v d) -> p d v", d=2),
                                pD[:].rearrange("p (d h v) -> p d h v",
                                                d=2, h=2)[:, :, 0, :],
                                act_fn)
                            nc.sync.dma_start(orows[half], outt[:])

    nc.compile()
    return nc


# --------------------------------------------------------------------------
# public entry point: full inputs in, full output out
# --------------------------------------------------------------------------

def kernel(x, weight, bias):
    global LAST_EXEC_NS
    x = np.ascontiguousarray(np.asarray(x, dtype=np.float32))
    consts = _host_constants(weight, bias)

    nc = build_nc()

    in_maps = []
    for core in range(NCORES):
        c0 = core * CPC
        xs = np.ascontiguousarray(
            x[:, c0:c0 + CPC].transpose(1, 0, 2, 3)).reshape(NIMG, N0, N0)
        in_maps.append({
            "x": xs,
            "khi": np.ascontiguousarray(consts["khi"][c0:c0 + CPC]),
            "klo": np.ascontiguousarray(consts["klo"][c0:c0 + CPC]),
            "khi2": np.ascontiguousarray(consts["khi2"][c0:c0 + CPC]),
            "klo2": np.ascontiguousarray(consts["klo2"][c0:c0 + CPC]),
            "gt": consts["gt"],
            "gbv": consts["gbv"],
            "csthi": consts["csthi"],
            "cstlo8": consts["cstlo8"],
            "rcs2": consts["rcs2"],
        })

    trace = os.environ.get("KERNEL_TRACE", "0") == "1"
    tmpdir = os.environ.get("KERNEL_TMPDIR") or None
    res = run_bass_kernel_spmd(nc, in_maps, list(range(NCORES)), trace=trace,
                               tmpdir=tmpdir)
    LAST_EXEC_NS = res.exec_time_ns

    out = np.empty((B, C, 2 * N0, 2 * N0), dtype=np.float32)
    for core in range(NCORES):
        c0 = core * CPC
        o = res.results[core]["out"].reshape(CPC, B, 2 * N0, 2 * N0)
        out[:, c0:c0 + CPC] = o.transpose(1, 0, 2, 3)
    return out
